# revision 13
# baseline (speedup 1.0000x reference)
"""Sliding-window causal attention (window=1024) for B=2,T=2048,H=16,D=128
on 8 trn2 NeuronCores. Shards the 32 (batch, head) pairs 4-per-core.

v4: host-side transposes (q/k passed [D, T] fp16, output stored as O^T and
transposed back on host). 1024-query groups: band blocks up to 1024 wide
stream as single f16 matmuls (st / pv / ones-denominator), cutting matmul
count to 72 per (b,h). Blocks pack widest+narrowest into double-bank PSUM
tiles with one exp per tile; normalization is a broadcast
reciprocal_approx_fast of the denominator bank applied pre-transpose.
"""
import math

import numpy as np
import ml_dtypes

import concourse.bass as bass
import concourse.bacc as bacc
import concourse.mybir as mybir
from concourse import tile
from concourse.bass_utils import run_bass_kernel_spmd

B, T, H, D = 2, 2048, 16, 128
WINDOW = 1024
NCORES = 8
BH = B * H                  # 32 (b,h) pairs
BH_PER_CORE = BH // NCORES  # 4
NT = T // 128               # 16 seq tiles
G = 4                       # q-tiles per group (512 queries)
NG = NT // G                # 4 groups
WB = WINDOW // 128          # window in blocks
GW = G * 128                # group width in queries (512)

f32 = mybir.dt.float32
f16 = mybir.dt.float16
bf16 = mybir.dt.bfloat16
AF = mybir.ActivationFunctionType
ALU = mybir.AluOpType


def band_blocks(g):
    """Key blocks intersecting group g's sliding band, with the trimmed
    q-tile range [t_min, t_max] each block must serve."""
    out = []
    for b in range(max(0, G * g - WB), G * g + G):
        t_min = max(G * g, b)
        t_max = min(G * g + G - 1, b + WB)
        if t_min <= t_max:
            out.append((b, t_min, t_max))
    return out


def pack_tiles(blocks):
    """Pack sorted-by-width blocks into (block, offset) tile groups whose
    widths sum to <= 1024, pairing widest with narrowest. Each tile gets one
    PSUM [128, 1024] allocation and one exp."""
    bs = sorted(blocks, key=lambda x: -(x[2] - x[1]))
    n = len(bs)
    tiles = []
    lo, hi = 0, n - 1
    while lo <= hi:
        b0 = bs[lo]
        w0 = (b0[2] - b0[1] + 1) * 128
        if lo < hi:
            b1 = bs[hi]
            w1 = (b1[2] - b1[1] + 1) * 128
            if w0 + w1 <= 1024:
                tiles.append([(b0, 0), (b1, max(w0, 512))])
                lo += 1
                hi -= 1
                continue
        tiles.append([(b0, 0)])
        lo += 1
    return tiles


def build_nc(n_bh=BH_PER_CORE):
    nc = bacc.Bacc()
    q = nc.declare_dram_parameter("q", [n_bh, T, D], f16, isOutput=False)
    k = nc.declare_dram_parameter("k", [n_bh, T, D], f16, isOutput=False)
    v = nc.declare_dram_parameter("v", [n_bh, T, D], f16, isOutput=False)
    o = nc.declare_dram_parameter("o", [n_bh, D, T], f16, isOutput=True)

    scale = 1.0 / math.sqrt(D)

    with tile.TileContext(nc) as tc:
        with (
            tc.tile_pool(name="const", bufs=1) as constp,
            tc.tile_pool(name="io", bufs=2) as iop,
            tc.tile_pool(name="es", bufs=6) as esp,
            tc.tile_pool(name="outp", bufs=2) as outp,
            tc.tile_pool(name="ps_st", bufs=2, space="PSUM") as ps_st,
            tc.tile_pool(name="ps_pv", bufs=2, space="PSUM") as ps_pv,
            tc.tile_pool(name="ps_sum", bufs=2, space="PSUM") as ps_sum,
        ):
            # split k/q loads across the two HWDGE queues so the transfers
            # run in parallel; halves let the first matmul start earlier
            def prefetch(bh):
                kth = [iop.tile([128, 1024], f16, tag=f"kt{h}",
                                name=f"kt{h}_{bh}") for h in range(2)]
                qth = [iop.tile([128, 1024], f16, tag=f"qt{h}",
                                name=f"qt{h}_{bh}") for h in range(2)]
                vb = iop.tile([128, NT, 128], f16, tag="vb", name=f"vb_{bh}")
                nc.sync.dma_start_transpose(
                    out=kth[0][:], in_=k[bh, 0:1024, :])
                nc.sync.dma_start_transpose(
                    out=qth[0][:], in_=q[bh, 0:1024, :])
                nc.sync.dma_start_transpose(
                    out=kth[1][:], in_=k[bh, 1024:2048, :])
                nc.sync.dma_start_transpose(
                    out=qth[1][:], in_=q[bh, 1024:2048, :])
                nc.sync.dma_start(
                    out=vb[:], in_=v[bh].rearrange("(n p) d -> p n d", p=128))
                return kth, qth, vb

            loaded = {0: prefetch(0)}

            # --- constants: ones for the denominator matmul, causal masks
            ones_f = constp.tile([128, 128], f32)
            mdiag_f = constp.tile([128, 128], f32)
            madiag_f = constp.tile([128, 128], f32)
            nc.gpsimd.memset(ones_f[:], 1.0)
            # diag mask (allowed k <= q): keep where col - p >= 0
            nc.gpsimd.affine_select(
                out=mdiag_f[:], in_=ones_f[:], compare_op=ALU.is_ge,
                fill=0.0, base=0, channel_multiplier=-1, pattern=[[1, 128]],
            )
            # anti-diag mask (allowed k > q): keep where p - col - 1 >= 0
            nc.gpsimd.affine_select(
                out=madiag_f[:], in_=ones_f[:], compare_op=ALU.is_ge,
                fill=0.0, base=-1, channel_multiplier=1, pattern=[[-1, 128]],
            )
            ones = constp.tile([128, 128], f16)
            mdiag = constp.tile([128, 128], f16)
            madiag = constp.tile([128, 128], f16)
            nc.vector.tensor_copy(ones[:], ones_f[:])
            nc.vector.tensor_copy(mdiag[:], mdiag_f[:])
            nc.vector.tensor_copy(madiag[:], madiag_f[:])

            def half_ap(th, c0, c1):
                h = c0 // 1024
                return th[h][:, c0 - 1024 * h:c1 - 1024 * h]

            for bh in range(n_bh):
                kth, qth, vb = loaded.pop(bh)
                if bh + 1 < n_bh:
                    loaded[bh + 1] = prefetch(bh + 1)

                for g in range(NG):
                    tiles = pack_tiles(band_blocks(g))
                    pv = ps_pv.tile([128, GW], f32, tag="pv")
                    sm = ps_sum.tile([128, GW], f32, tag="sm")

                    # f32 PSUM writes max out at one bank (512 cols): split
                    # each block's pv/sm range at absolute 512 boundaries.
                    # Emission order = tile order, so per-bank first/last
                    # flags are known up front; the widest block is emitted
                    # first and fully covers every bank it touches.
                    def pieces_of(tl):
                        out = []
                        for (b, t_min, t_max), eo in tl:
                            w = (t_max - t_min + 1) * 128
                            off = (t_min - G * g) * 128
                            a = off
                            while a < off + w:
                                bnd = min(off + w, (a // 512 + 1) * 512)
                                out.append((b, a, bnd, eo + a - off))
                                a = bnd
                        return out

                    all_pieces = [pieces_of(tl) for tl in tiles]
                    flat = [p for tp in all_pieces for p in tp]
                    first_in_bank = {}
                    last_in_bank = {}
                    for i, (b, a, bnd, ea) in enumerate(flat):
                        bank = a // 512
                        first_in_bank.setdefault(bank, i)
                        last_in_bank[bank] = i
                    pidx = [0]

                    def emit_pvsm(es, tp):
                        for b, a, bnd, ea in tp:
                            i = pidx[0]
                            bank = a // 512
                            st_f = first_in_bank[bank] == i
                            sp_f = last_in_bank[bank] == i
                            nc.tensor.matmul(
                                pv[:, a:bnd], vb[:, b, :],
                                es[:, ea:ea + bnd - a],
                                start=st_f, stop=sp_f)
                            nc.tensor.matmul(
                                sm[:, a:bnd], ones[:],
                                es[:, ea:ea + bnd - a],
                                start=st_f, stop=sp_f)
                            pidx[0] += 1

                    pending = None
                    for ti, tl in enumerate(tiles):
                        stp = ps_st.tile([128, 1024], f32, tag="st")
                        es = esp.tile([128, 1024], f16, tag="es")
                        wtot = 0
                        for (b, t_min, t_max), eo in tl:
                            w = (t_max - t_min + 1) * 128
                            # q halves never split a block's q-range (ranges
                            # stay within one 1024-aligned group half)
                            # split st at the stp tile's bank boundary too
                            c = 0
                            while c < w:
                                cw = min(w - c, ((eo + c) // 512 + 1) * 512
                                         - (eo + c))
                                nc.tensor.matmul(
                                    stp[:, eo + c:eo + c + cw],
                                    half_ap(kth, 128 * b, 128 * b + 128),
                                    half_ap(qth, 128 * t_min + c,
                                            128 * t_min + c + cw),
                                    start=True, stop=True)
                                c += cw
                            wtot = eo + w
                        nc.scalar.activation(
                            es[:, 0:wtot], stp[:, 0:wtot], AF.Exp,
                            scale=scale)
                        for (b, t_min, t_max), eo in tl:
                            w = (t_max - t_min + 1) * 128
                            if b >= G * g:
                                nc.vector.tensor_mul(
                                    es[:, eo:eo + 128], es[:, eo:eo + 128],
                                    mdiag[:])
                            if b + WB <= G * g + G - 1:
                                nc.gpsimd.tensor_mul(
                                    es[:, eo + w - 128:eo + w],
                                    es[:, eo + w - 128:eo + w], madiag[:])
                        if pending is not None:
                            emit_pvsm(*pending)
                        pending = (es, all_pieces[ti])
                    emit_pvsm(*pending)

                    # --- normalize pre-transpose in halves (pipelines the
                    # reciprocal, multiply and store), store O^T
                    for h in range(GW // 512):
                        c = 512 * h
                        rec = outp.tile([128, 512], f32, tag=f"rec{h}")
                        nc.vector.reciprocal_approx_fast(
                            rec[:], sm[:, c:c + 512])
                        otn = outp.tile([128, 512], f16, tag=f"otn{h}")
                        nc.vector.tensor_mul(otn[:], pv[:, c:c + 512], rec[:])
                        nc.sync.dma_start(
                            out=o[bh, :, GW * g + c:GW * g + c + 512],
                            in_=otn[:])


    if not nc.is_finalized():
        nc.finalize()
    return nc


_nc = None


def _get_nc():
    global _nc
    if _nc is None:
        _nc = build_nc()
    return _nc


def make_in_maps(q, k, v):
    # [B, T, H, D] -> [B*H, T, D] fp16; q/k additionally pre-transposed
    # to [B*H, D, T] (kernel wants the [d, t] layout)
    qs = np.ascontiguousarray(
        np.asarray(q, dtype=np.float32).transpose(0, 2, 1, 3)
        .reshape(BH, T, D)).astype(np.float16)
    ks = np.ascontiguousarray(
        np.asarray(k, dtype=np.float32).transpose(0, 2, 1, 3)
        .reshape(BH, T, D)).astype(np.float16)
    vs = np.ascontiguousarray(
        np.asarray(v, dtype=np.float32).transpose(0, 2, 1, 3)
        .reshape(BH, T, D)).astype(np.float16)
    return [
        {
            "q": qs[c * BH_PER_CORE:(c + 1) * BH_PER_CORE],
            "k": ks[c * BH_PER_CORE:(c + 1) * BH_PER_CORE],
            "v": vs[c * BH_PER_CORE:(c + 1) * BH_PER_CORE],
        }
        for c in range(NCORES)
    ]


def assemble_out(results):
    # results hold O^T [n_bh, D, T] fp16 -> [BH, T, D] f32
    out = np.empty((BH, T, D), np.float32)
    for c in range(NCORES):
        ot = np.asarray(results[c]["o"], dtype=np.float32)  # [n_bh, D, T]
        out[c * BH_PER_CORE:(c + 1) * BH_PER_CORE] = ot.transpose(0, 2, 1)
    return np.ascontiguousarray(
        out.reshape(B, H, T, D).transpose(0, 2, 1, 3))


def kernel(q, k, v, window_size):
    assert int(window_size) == WINDOW
    in_maps = make_in_maps(q, k, v)
    res = run_bass_kernel_spmd(_get_nc(), in_maps, list(range(NCORES))).results
    return assemble_out(res)


# revision 19
# speedup vs baseline: 1.1047x; 1.1047x over previous
"""Sliding-window causal attention (window=1024) for B=2,T=2048,H=16,D=128
on 8 trn2 NeuronCores. Shards the 32 (batch, head) pairs 4-per-core.

v4: host-side transposes (q/k passed [D, T] fp16, output stored as O^T and
transposed back on host). 1024-query groups: band blocks up to 1024 wide
stream as single f16 matmuls (st / pv / ones-denominator), cutting matmul
count to 72 per (b,h). Blocks pack widest+narrowest into double-bank PSUM
tiles with one exp per tile; normalization is a broadcast
reciprocal_approx_fast of the denominator bank applied pre-transpose.
"""
import math

import numpy as np
import ml_dtypes

import concourse.bass as bass
import concourse.bacc as bacc
import concourse.mybir as mybir
from concourse import tile
from concourse.bass_utils import run_bass_kernel_spmd

B, T, H, D = 2, 2048, 16, 128
WINDOW = 1024
NCORES = 8
BH = B * H                  # 32 (b,h) pairs
BH_PER_CORE = BH // NCORES  # 4
NT = T // 128               # 16 seq tiles
G = 4                       # q-tiles per group (512 queries)
NG = NT // G                # 4 groups
WB = WINDOW // 128          # window in blocks
GW = G * 128                # group width in queries (512)

f32 = mybir.dt.float32
f16 = mybir.dt.float16
bf16 = mybir.dt.bfloat16
AF = mybir.ActivationFunctionType
ALU = mybir.AluOpType


def band_blocks(g):
    """Key blocks intersecting group g's sliding band, with the trimmed
    q-tile range [t_min, t_max] each block must serve."""
    out = []
    for b in range(max(0, G * g - WB), G * g + G):
        t_min = max(G * g, b)
        t_max = min(G * g + G - 1, b + WB)
        if t_min <= t_max:
            out.append((b, t_min, t_max))
    return out


def pack_tiles(blocks):
    """Pack sorted-by-width blocks into (block, offset) tile groups whose
    widths sum to <= 1024, pairing widest with narrowest. Each tile gets one
    PSUM [128, 1024] allocation and one exp."""
    bs = sorted(blocks, key=lambda x: -(x[2] - x[1]))
    n = len(bs)
    tiles = []
    lo, hi = 0, n - 1
    while lo <= hi:
        b0 = bs[lo]
        w0 = (b0[2] - b0[1] + 1) * 128
        if lo < hi:
            b1 = bs[hi]
            w1 = (b1[2] - b1[1] + 1) * 128
            if w0 + w1 <= 1024:
                tiles.append([(b0, 0), (b1, max(w0, 512))])
                lo += 1
                hi -= 1
                continue
        tiles.append([(b0, 0)])
        lo += 1
    return tiles


def build_nc(n_bh=BH_PER_CORE):
    nc = bacc.Bacc()
    q = nc.declare_dram_parameter("q", [n_bh, T, D], f16, isOutput=False)
    k = nc.declare_dram_parameter("k", [n_bh, T, D], f16, isOutput=False)
    v = nc.declare_dram_parameter("v", [n_bh, T, D], f16, isOutput=False)
    o = nc.declare_dram_parameter("o", [n_bh, D, T], f16, isOutput=True)

    scale = 1.0 / math.sqrt(D)

    with tile.TileContext(nc) as tc:
        with (
            tc.tile_pool(name="const", bufs=1) as constp,
            tc.tile_pool(name="io", bufs=2) as iop,
            tc.tile_pool(name="es", bufs=6) as esp,
            tc.tile_pool(name="outp", bufs=2) as outp,
            tc.tile_pool(name="ps_st", bufs=2, space="PSUM") as ps_st,
            tc.tile_pool(name="ps_pv", bufs=2, space="PSUM") as ps_pv,
            tc.tile_pool(name="ps_sum", bufs=2, space="PSUM") as ps_sum,
        ):
            # split k/q loads across the two HWDGE queues so the transfers
            # run in parallel; halves let the first matmul start earlier
            # piece layout: quarters for the first 1024 t (lets the first
            # matmuls start as soon as a 512-col transpose lands), one half
            # for the rest
            PIECES = [(0, 1024), (1024, 2048)]

            def prefetch(bh):
                kts = [iop.tile([128, b - a], f16, tag=f"kt{i}",
                                name=f"kt{i}_{bh}")
                       for i, (a, b) in enumerate(PIECES)]
                qts = [iop.tile([128, b - a], f16, tag=f"qt{i}",
                                name=f"qt{i}_{bh}")
                       for i, (a, b) in enumerate(PIECES)]
                vb = iop.tile([128, NT, 128], f16, tag="vb", name=f"vb_{bh}")
                for i, (a, b) in enumerate(PIECES):
                    nc.sync.dma_start_transpose(
                        out=kts[i][:], in_=k[bh, a:b, :])
                    nc.sync.dma_start_transpose(
                        out=qts[i][:], in_=q[bh, a:b, :])
                nc.sync.dma_start(
                    out=vb[:], in_=v[bh].rearrange("(n p) d -> p n d", p=128))
                return kts, qts, vb

            loaded = {0: prefetch(0)}

            # --- constants: ones for the denominator matmul, causal masks
            ones_f = constp.tile([128, 128], f32)
            mdiag_f = constp.tile([128, 128], f32)
            madiag_f = constp.tile([128, 128], f32)
            nc.gpsimd.memset(ones_f[:], 1.0)
            # diag mask (allowed k <= q): keep where col - p >= 0
            nc.gpsimd.affine_select(
                out=mdiag_f[:], in_=ones_f[:], compare_op=ALU.is_ge,
                fill=0.0, base=0, channel_multiplier=-1, pattern=[[1, 128]],
            )
            # anti-diag mask (allowed k > q): keep where p - col - 1 >= 0
            nc.gpsimd.affine_select(
                out=madiag_f[:], in_=ones_f[:], compare_op=ALU.is_ge,
                fill=0.0, base=-1, channel_multiplier=1, pattern=[[-1, 128]],
            )
            ones = constp.tile([128, 128], f16)
            mdiag = constp.tile([128, 128], f16)
            madiag = constp.tile([128, 128], f16)
            nc.vector.tensor_copy(ones[:], ones_f[:])
            nc.vector.tensor_copy(mdiag[:], mdiag_f[:])
            nc.vector.tensor_copy(madiag[:], madiag_f[:])

            def half_ap(ts, c0, c1):
                for i, (a, b) in enumerate(PIECES):
                    if a <= c0 and c1 <= b:
                        return ts[i][:, c0 - a:c1 - a]
                raise AssertionError(f"range [{c0},{c1}) crosses pieces")

            for bh in range(n_bh):
                kts, qts, vb = loaded.pop(bh)
                if bh + 1 < n_bh:
                    loaded[bh + 1] = prefetch(bh + 1)

                for g in range(NG):
                    tiles = pack_tiles(band_blocks(g))
                    pv = ps_pv.tile([128, GW], f32, tag="pv")
                    sm = ps_sum.tile([128, GW], f32, tag="sm")

                    # f32 PSUM writes max out at one bank (512 cols): split
                    # each block's pv/sm range at absolute 512 boundaries.
                    # Emission order = tile order, so per-bank first/last
                    # flags are known up front; the widest block is emitted
                    # first and fully covers every bank it touches.
                    def pieces_of(tl):
                        out = []
                        for (b, t_min, t_max), eo in tl:
                            w = (t_max - t_min + 1) * 128
                            off = (t_min - G * g) * 128
                            a = off
                            while a < off + w:
                                bnd = min(off + w, (a // 512 + 1) * 512)
                                out.append((b, a, bnd, eo + a - off))
                                a = bnd
                        return out

                    all_pieces = [pieces_of(tl) for tl in tiles]
                    flat = [p for tp in all_pieces for p in tp]
                    first_in_bank = {}
                    last_in_bank = {}
                    for i, (b, a, bnd, ea) in enumerate(flat):
                        bank = a // 512
                        first_in_bank.setdefault(bank, i)
                        last_in_bank[bank] = i
                    pidx = [0]

                    def emit_pvsm(es, tp):
                        for b, a, bnd, ea in tp:
                            i = pidx[0]
                            bank = a // 512
                            st_f = first_in_bank[bank] == i
                            sp_f = last_in_bank[bank] == i
                            nc.tensor.matmul(
                                pv[:, a:bnd], vb[:, b, :],
                                es[:, ea:ea + bnd - a],
                                start=st_f, stop=sp_f)
                            nc.tensor.matmul(
                                sm[:, a:bnd], ones[:],
                                es[:, ea:ea + bnd - a],
                                start=st_f, stop=sp_f)
                            pidx[0] += 1

                    pending = None
                    for ti, tl in enumerate(tiles):
                        stp = ps_st.tile([128, 1024], f32, tag="st")
                        es = esp.tile([128, 1024], f16, tag="es")
                        wtot = 0
                        for (b, t_min, t_max), eo in tl:
                            w = (t_max - t_min + 1) * 128
                            # q halves never split a block's q-range (ranges
                            # stay within one 1024-aligned group half)
                            # split st at the stp tile's bank boundary too
                            c = 0
                            while c < w:
                                cw = min(w - c, ((eo + c) // 512 + 1) * 512
                                         - (eo + c))
                                nc.tensor.matmul(
                                    stp[:, eo + c:eo + c + cw],
                                    half_ap(kts, 128 * b, 128 * b + 128),
                                    half_ap(qts, 128 * t_min + c,
                                            128 * t_min + c + cw),
                                    start=True, stop=True)
                                c += cw
                            wtot = eo + w
                        nc.scalar.activation(
                            es[:, 0:wtot], stp[:, 0:wtot], AF.Exp,
                            scale=scale)
                        for (b, t_min, t_max), eo in tl:
                            w = (t_max - t_min + 1) * 128
                            if b >= G * g:
                                nc.vector.tensor_mul(
                                    es[:, eo:eo + 128], es[:, eo:eo + 128],
                                    mdiag[:])
                            if b + WB <= G * g + G - 1:
                                nc.gpsimd.tensor_mul(
                                    es[:, eo + w - 128:eo + w],
                                    es[:, eo + w - 128:eo + w], madiag[:])
                        if pending is not None:
                            emit_pvsm(*pending)
                        pending = (es, all_pieces[ti])
                    emit_pvsm(*pending)

                    # --- normalize pre-transpose in halves (pipelines the
                    # reciprocal, multiply and store), store O^T
                    for h in range(GW // 256):
                        c = 256 * h
                        rec = outp.tile([128, 256], f32, tag=f"rec{h}")
                        nc.vector.reciprocal_approx_fast(
                            rec[:], sm[:, c:c + 256])
                        otn = outp.tile([128, 256], f16, tag=f"otn{h}")
                        nc.vector.tensor_mul(otn[:], pv[:, c:c + 256], rec[:])
                        nc.sync.dma_start(
                            out=o[bh, :, GW * g + c:GW * g + c + 256],
                            in_=otn[:])


    if not nc.is_finalized():
        nc.finalize()
    return nc


_nc = None


def _get_nc():
    global _nc
    if _nc is None:
        _nc = build_nc()
    return _nc


def make_in_maps(q, k, v):
    # [B, T, H, D] -> [B*H, T, D] fp16; q/k additionally pre-transposed
    # to [B*H, D, T] (kernel wants the [d, t] layout)
    qs = np.ascontiguousarray(
        np.asarray(q, dtype=np.float32).transpose(0, 2, 1, 3)
        .reshape(BH, T, D)).astype(np.float16)
    ks = np.ascontiguousarray(
        np.asarray(k, dtype=np.float32).transpose(0, 2, 1, 3)
        .reshape(BH, T, D)).astype(np.float16)
    vs = np.ascontiguousarray(
        np.asarray(v, dtype=np.float32).transpose(0, 2, 1, 3)
        .reshape(BH, T, D)).astype(np.float16)
    return [
        {
            "q": qs[c * BH_PER_CORE:(c + 1) * BH_PER_CORE],
            "k": ks[c * BH_PER_CORE:(c + 1) * BH_PER_CORE],
            "v": vs[c * BH_PER_CORE:(c + 1) * BH_PER_CORE],
        }
        for c in range(NCORES)
    ]


def assemble_out(results):
    # results hold O^T [n_bh, D, T] fp16 -> [BH, T, D] f32
    out = np.empty((BH, T, D), np.float32)
    for c in range(NCORES):
        ot = np.asarray(results[c]["o"], dtype=np.float32)  # [n_bh, D, T]
        out[c * BH_PER_CORE:(c + 1) * BH_PER_CORE] = ot.transpose(0, 2, 1)
    return np.ascontiguousarray(
        out.reshape(B, H, T, D).transpose(0, 2, 1, 3))


def kernel(q, k, v, window_size):
    assert int(window_size) == WINDOW
    in_maps = make_in_maps(q, k, v)
    res = run_bass_kernel_spmd(_get_nc(), in_maps, list(range(NCORES))).results
    return assemble_out(res)


# revision 24
# speedup vs baseline: 1.1354x; 1.0279x over previous
"""Sliding-window causal attention (window=1024) for B=2,T=2048,H=16,D=128
on 8 trn2 NeuronCores. Shards the 32 (batch, head) pairs 4-per-core.

Inputs are fp16 (host-converted); q/k load pre-transposed to [d, t] via the
xbar DMA-transpose engine in 1024-col halves (fewer, larger transfers win on
the serialized transpose channel; next-bh prefetch is issued at bh start).
Per 512-query group, the 128x128 band blocks pack widest+narrowest into
double-bank PSUM tiles: S^T = K @ Q^T, one wide exp on the scalar engine
(fixed 352-cycle overhead amortized), causal trim masks split across DVE and
GpSimd, then PV and the ones-matmul softmax denominators accumulate in PSUM
(the denominator pass is unavoidable: normalize+contract needs either a
partition-sum or a layout flip, each one extra PE pass). Normalization
multiplies by a broadcast reciprocal_approx_fast of the denominator bank
pre-transpose; O^T stores as fp16 [D, T] and the host transposes back.
"""
import math

import numpy as np
import ml_dtypes

import concourse.bass as bass
import concourse.bacc as bacc
import concourse.mybir as mybir
from concourse import tile
from concourse.bass_utils import run_bass_kernel_spmd

B, T, H, D = 2, 2048, 16, 128
WINDOW = 1024
NCORES = 8
BH = B * H                  # 32 (b,h) pairs
BH_PER_CORE = BH // NCORES  # 4
NT = T // 128               # 16 seq tiles
G = 4                       # q-tiles per group (512 queries)
NG = NT // G                # 4 groups
WB = WINDOW // 128          # window in blocks
GW = G * 128                # group width in queries (512)

f32 = mybir.dt.float32
f16 = mybir.dt.float16
bf16 = mybir.dt.bfloat16
AF = mybir.ActivationFunctionType
ALU = mybir.AluOpType


def band_blocks(g):
    """Key blocks intersecting group g's sliding band, with the trimmed
    q-tile range [t_min, t_max] each block must serve."""
    out = []
    for b in range(max(0, G * g - WB), G * g + G):
        t_min = max(G * g, b)
        t_max = min(G * g + G - 1, b + WB)
        if t_min <= t_max:
            out.append((b, t_min, t_max))
    return out


def pack_tiles(blocks):
    """Pack sorted-by-width blocks into (block, offset) tile groups whose
    widths sum to <= 1024, pairing widest with narrowest. Each tile gets one
    PSUM [128, 1024] allocation and one exp."""
    bs = sorted(blocks, key=lambda x: -(x[2] - x[1]))
    n = len(bs)
    tiles = []
    lo, hi = 0, n - 1
    while lo <= hi:
        b0 = bs[lo]
        w0 = (b0[2] - b0[1] + 1) * 128
        if lo < hi:
            b1 = bs[hi]
            w1 = (b1[2] - b1[1] + 1) * 128
            if w0 + w1 <= 1024:
                tiles.append([(b0, 0), (b1, max(w0, 512))])
                lo += 1
                hi -= 1
                continue
        tiles.append([(b0, 0)])
        lo += 1
    return tiles


def build_nc(n_bh=BH_PER_CORE):
    nc = bacc.Bacc()
    q = nc.declare_dram_parameter("q", [n_bh, T, D], f16, isOutput=False)
    k = nc.declare_dram_parameter("k", [n_bh, T, D], f16, isOutput=False)
    v = nc.declare_dram_parameter("v", [n_bh, T, D], f16, isOutput=False)
    o = nc.declare_dram_parameter("o", [n_bh, D, T], f16, isOutput=True)

    scale = 1.0 / math.sqrt(D)

    with tile.TileContext(nc) as tc:
        with (
            tc.tile_pool(name="const", bufs=1) as constp,
            tc.tile_pool(name="io", bufs=2) as iop,
            tc.tile_pool(name="es", bufs=6) as esp,
            tc.tile_pool(name="outp", bufs=2) as outp,
            tc.tile_pool(name="ps_st", bufs=2, space="PSUM") as ps_st,
            tc.tile_pool(name="ps_pv", bufs=2, space="PSUM") as ps_pv,
            tc.tile_pool(name="ps_sum", bufs=2, space="PSUM") as ps_sum,
        ):
            # piece layout: quarters for the first 1024 t (lets the first
            # matmuls start as soon as a 512-col transpose lands), one half
            # for the rest
            PIECES = [(0, 1024), (1024, 2048)]

            def prefetch(bh):
                kts = [iop.tile([128, b - a], f16, tag=f"kt{i}",
                                name=f"kt{i}_{bh}")
                       for i, (a, b) in enumerate(PIECES)]
                qts = [iop.tile([128, b - a], f16, tag=f"qt{i}",
                                name=f"qt{i}_{bh}")
                       for i, (a, b) in enumerate(PIECES)]
                vb = iop.tile([128, NT, 128], f16, tag="vb", name=f"vb_{bh}")
                for i, (a, b) in enumerate(PIECES):
                    nc.sync.dma_start_transpose(
                        out=kts[i][:], in_=k[bh, a:b, :])
                    nc.sync.dma_start_transpose(
                        out=qts[i][:], in_=q[bh, a:b, :])
                nc.sync.dma_start(
                    out=vb[:], in_=v[bh].rearrange("(n p) d -> p n d", p=128))
                return kts, qts, vb

            loaded = {0: prefetch(0)}

            # --- constants: ones for the denominator matmul, causal masks
            ones_f = constp.tile([128, 128], f32)
            mdiag_f = constp.tile([128, 128], f32)
            madiag_f = constp.tile([128, 128], f32)
            nc.gpsimd.memset(ones_f[:], 1.0)
            # diag mask (allowed k <= q): keep where col - p >= 0
            nc.gpsimd.affine_select(
                out=mdiag_f[:], in_=ones_f[:], compare_op=ALU.is_ge,
                fill=0.0, base=0, channel_multiplier=-1, pattern=[[1, 128]],
            )
            # anti-diag mask (allowed k > q): keep where p - col - 1 >= 0
            nc.gpsimd.affine_select(
                out=madiag_f[:], in_=ones_f[:], compare_op=ALU.is_ge,
                fill=0.0, base=-1, channel_multiplier=1, pattern=[[-1, 128]],
            )
            ones = constp.tile([128, 128], f16)
            mdiag = constp.tile([128, 128], f16)
            madiag = constp.tile([128, 128], f16)
            nc.vector.tensor_copy(ones[:], ones_f[:])
            nc.vector.tensor_copy(mdiag[:], mdiag_f[:])
            nc.vector.tensor_copy(madiag[:], madiag_f[:])

            def half_ap(ts, c0, c1):
                for i, (a, b) in enumerate(PIECES):
                    if a <= c0 and c1 <= b:
                        return ts[i][:, c0 - a:c1 - a]
                raise AssertionError(f"range [{c0},{c1}) crosses pieces")

            for bh in range(n_bh):
                kts, qts, vb = loaded.pop(bh)
                if bh + 1 < n_bh:
                    loaded[bh + 1] = prefetch(bh + 1)

                for g in range(NG):
                    tiles = pack_tiles(band_blocks(g))
                    pv = ps_pv.tile([128, GW], f32, tag="pv")
                    sm = ps_sum.tile([128, GW], f32, tag="sm")

                    # f32 PSUM writes max out at one bank (512 cols): split
                    # each block's pv/sm range at absolute 512 boundaries.
                    # Emission order = tile order, so per-bank first/last
                    # flags are known up front; the widest block is emitted
                    # first and fully covers every bank it touches.
                    def pieces_of(tl):
                        out = []
                        for (b, t_min, t_max), eo in tl:
                            w = (t_max - t_min + 1) * 128
                            off = (t_min - G * g) * 128
                            a = off
                            while a < off + w:
                                bnd = min(off + w, (a // 512 + 1) * 512)
                                out.append((b, a, bnd, eo + a - off))
                                a = bnd
                        return out

                    all_pieces = [pieces_of(tl) for tl in tiles]
                    flat = [p for tp in all_pieces for p in tp]
                    first_in_bank = {}
                    last_in_bank = {}
                    for i, (b, a, bnd, ea) in enumerate(flat):
                        bank = a // 512
                        first_in_bank.setdefault(bank, i)
                        last_in_bank[bank] = i
                    pidx = [0]

                    def emit_pvsm(es, tp):
                        for b, a, bnd, ea in tp:
                            i = pidx[0]
                            bank = a // 512
                            st_f = first_in_bank[bank] == i
                            sp_f = last_in_bank[bank] == i
                            nc.tensor.matmul(
                                pv[:, a:bnd], vb[:, b, :],
                                es[:, ea:ea + bnd - a],
                                start=st_f, stop=sp_f)
                            nc.tensor.matmul(
                                sm[:, a:bnd], ones[:],
                                es[:, ea:ea + bnd - a],
                                start=st_f, stop=sp_f)
                            pidx[0] += 1

                    pending = None
                    for ti, tl in enumerate(tiles):
                        stp = ps_st.tile([128, 1024], f32, tag="st")
                        es = esp.tile([128, 1024], f16, tag="es")
                        wtot = 0
                        for (b, t_min, t_max), eo in tl:
                            w = (t_max - t_min + 1) * 128
                            # split st at the stp tile's bank boundary
                            c = 0
                            while c < w:
                                cw = min(w - c, ((eo + c) // 512 + 1) * 512
                                         - (eo + c))
                                nc.tensor.matmul(
                                    stp[:, eo + c:eo + c + cw],
                                    half_ap(kts, 128 * b, 128 * b + 128),
                                    half_ap(qts, 128 * t_min + c,
                                            128 * t_min + c + cw),
                                    start=True, stop=True)
                                c += cw
                            wtot = eo + w
                        nc.scalar.activation(
                            es[:, 0:wtot], stp[:, 0:wtot], AF.Exp,
                            scale=scale)
                        for (b, t_min, t_max), eo in tl:
                            w = (t_max - t_min + 1) * 128
                            if b >= G * g:
                                nc.vector.tensor_mul(
                                    es[:, eo:eo + 128], es[:, eo:eo + 128],
                                    mdiag[:])
                            if b + WB <= G * g + G - 1:
                                nc.gpsimd.tensor_mul(
                                    es[:, eo + w - 128:eo + w],
                                    es[:, eo + w - 128:eo + w], madiag[:])
                        if pending is not None:
                            emit_pvsm(*pending)
                        pending = (es, all_pieces[ti])
                    emit_pvsm(*pending)

                    # --- normalize pre-transpose in halves (pipelines the
                    # reciprocal, multiply and store), store O^T
                    for h in range(GW // 256):
                        c = 256 * h
                        rec = outp.tile([128, 256], f32, tag=f"rec{h}")
                        nc.vector.reciprocal_approx_fast(
                            rec[:], sm[:, c:c + 256])
                        otn = outp.tile([128, 256], f16, tag=f"otn{h}")
                        nc.vector.tensor_mul(otn[:], pv[:, c:c + 256], rec[:])
                        nc.sync.dma_start(
                            out=o[bh, :, GW * g + c:GW * g + c + 256],
                            in_=otn[:])


    if not nc.is_finalized():
        nc.finalize()
    return nc


_nc = None


def _get_nc():
    global _nc
    if _nc is None:
        _nc = build_nc()
    return _nc


def make_in_maps(q, k, v):
    # [B, T, H, D] -> [B*H, T, D] fp16; q/k additionally pre-transposed
    # to [B*H, D, T] (kernel wants the [d, t] layout)
    qs = np.ascontiguousarray(
        np.asarray(q, dtype=np.float32).transpose(0, 2, 1, 3)
        .reshape(BH, T, D)).astype(np.float16)
    ks = np.ascontiguousarray(
        np.asarray(k, dtype=np.float32).transpose(0, 2, 1, 3)
        .reshape(BH, T, D)).astype(np.float16)
    vs = np.ascontiguousarray(
        np.asarray(v, dtype=np.float32).transpose(0, 2, 1, 3)
        .reshape(BH, T, D)).astype(np.float16)
    return [
        {
            "q": qs[c * BH_PER_CORE:(c + 1) * BH_PER_CORE],
            "k": ks[c * BH_PER_CORE:(c + 1) * BH_PER_CORE],
            "v": vs[c * BH_PER_CORE:(c + 1) * BH_PER_CORE],
        }
        for c in range(NCORES)
    ]


def assemble_out(results):
    # results hold O^T [n_bh, D, T] fp16 -> [BH, T, D] f32
    out = np.empty((BH, T, D), np.float32)
    for c in range(NCORES):
        ot = np.asarray(results[c]["o"], dtype=np.float32)  # [n_bh, D, T]
        out[c * BH_PER_CORE:(c + 1) * BH_PER_CORE] = ot.transpose(0, 2, 1)
    return np.ascontiguousarray(
        out.reshape(B, H, T, D).transpose(0, 2, 1, 3))


def kernel(q, k, v, window_size):
    assert int(window_size) == WINDOW
    in_maps = make_in_maps(q, k, v)
    res = run_bass_kernel_spmd(_get_nc(), in_maps, list(range(NCORES))).results
    return assemble_out(res)


# revision 26
# speedup vs baseline: 1.1454x; 1.0088x over previous
"""Sliding-window causal attention (window=1024) for B=2,T=2048,H=16,D=128
on 8 trn2 NeuronCores. Shards the 32 (batch, head) pairs 4-per-core.

Inputs are fp16 (host-converted); q/k load pre-transposed to [d, t] via the
xbar DMA-transpose engine in 1024-col halves (fewer, larger transfers win on
the serialized transpose channel; next-bh prefetch is issued at bh start).
Per 512-query group, the 128x128 band blocks pack widest+narrowest into
double-bank PSUM tiles: S^T = K @ Q^T, one wide exp on the scalar engine
(fixed 352-cycle overhead amortized), causal trim masks split across DVE and
GpSimd, then PV and the ones-matmul softmax denominators accumulate in PSUM
(the denominator pass is unavoidable: normalize+contract needs either a
partition-sum or a layout flip, each one extra PE pass). Normalization
multiplies by a broadcast reciprocal_approx_fast of the denominator bank
pre-transpose; O^T stores as fp16 [D, T] and the host transposes back.
"""
import math

import numpy as np
import ml_dtypes

import concourse.bass as bass
import concourse.bacc as bacc
import concourse.mybir as mybir
from concourse import tile
from concourse.bass_utils import run_bass_kernel_spmd

B, T, H, D = 2, 2048, 16, 128
WINDOW = 1024
NCORES = 8
BH = B * H                  # 32 (b,h) pairs
BH_PER_CORE = BH // NCORES  # 4
NT = T // 128               # 16 seq tiles
G = 4                       # q-tiles per group (512 queries)
NG = NT // G                # 4 groups
WB = WINDOW // 128          # window in blocks
GW = G * 128                # group width in queries (512)

f32 = mybir.dt.float32
f16 = mybir.dt.float16
bf16 = mybir.dt.bfloat16
AF = mybir.ActivationFunctionType
ALU = mybir.AluOpType


def band_blocks(g):
    """Key blocks intersecting group g's sliding band, with the trimmed
    q-tile range [t_min, t_max] each block must serve."""
    out = []
    for b in range(max(0, G * g - WB), G * g + G):
        t_min = max(G * g, b)
        t_max = min(G * g + G - 1, b + WB)
        if t_min <= t_max:
            out.append((b, t_min, t_max))
    return out


def pack_tiles(blocks):
    """Pack sorted-by-width blocks into (block, offset) tile groups whose
    widths sum to <= 1024, pairing widest with narrowest. Each tile gets one
    PSUM [128, 1024] allocation and one exp."""
    bs = sorted(blocks, key=lambda x: -(x[2] - x[1]))
    n = len(bs)
    tiles = []
    lo, hi = 0, n - 1
    while lo <= hi:
        b0 = bs[lo]
        w0 = (b0[2] - b0[1] + 1) * 128
        if lo < hi:
            b1 = bs[hi]
            w1 = (b1[2] - b1[1] + 1) * 128
            if w0 + w1 <= 1024:
                tiles.append([(b0, 0), (b1, max(w0, 512))])
                lo += 1
                hi -= 1
                continue
        tiles.append([(b0, 0)])
        lo += 1
    return tiles


def build_nc(n_bh=BH_PER_CORE):
    nc = bacc.Bacc()
    q = nc.declare_dram_parameter("q", [n_bh, T, D], f16, isOutput=False)
    k = nc.declare_dram_parameter("k", [n_bh, T, D], f16, isOutput=False)
    v = nc.declare_dram_parameter("v", [n_bh, T, D], f16, isOutput=False)
    o = nc.declare_dram_parameter("o", [n_bh, D, T], f16, isOutput=True)

    scale = 1.0 / math.sqrt(D)

    with tile.TileContext(nc) as tc:
        with (
            tc.tile_pool(name="const", bufs=1) as constp,
            tc.tile_pool(name="io", bufs=2) as iop,
            tc.tile_pool(name="es", bufs=6) as esp,
            tc.tile_pool(name="outp", bufs=2) as outp,
            tc.tile_pool(name="ps_st", bufs=2, space="PSUM") as ps_st,
            tc.tile_pool(name="ps_pv", bufs=2, space="PSUM") as ps_pv,
            tc.tile_pool(name="ps_sum", bufs=2, space="PSUM") as ps_sum,
        ):
            # piece layout: quarters for the first 1024 t (lets the first
            # matmuls start as soon as a 512-col transpose lands), one half
            # for the rest
            PIECES = [(0, 1024), (1024, 2048)]

            def prefetch(bh):
                kts = [iop.tile([128, b - a], f16, tag=f"kt{i}",
                                name=f"kt{i}_{bh}")
                       for i, (a, b) in enumerate(PIECES)]
                qts = [iop.tile([128, b - a], f16, tag=f"qt{i}",
                                name=f"qt{i}_{bh}")
                       for i, (a, b) in enumerate(PIECES)]
                vb = iop.tile([128, NT, 128], f16, tag="vb", name=f"vb_{bh}")
                for i, (a, b) in enumerate(PIECES):
                    nc.sync.dma_start_transpose(
                        out=kts[i][:], in_=k[bh, a:b, :])
                    nc.sync.dma_start_transpose(
                        out=qts[i][:], in_=q[bh, a:b, :])
                nc.sync.dma_start(
                    out=vb[:], in_=v[bh].rearrange("(n p) d -> p n d", p=128))
                return kts, qts, vb

            loaded = {0: prefetch(0)}

            # --- constants: ones for the denominator matmul, causal masks
            ones_f = constp.tile([128, 128], f32)
            mdiag_f = constp.tile([128, 128], f32)
            madiag_f = constp.tile([128, 128], f32)
            nc.gpsimd.memset(ones_f[:], 1.0)
            # diag mask (allowed k <= q): keep where col - p >= 0
            nc.gpsimd.affine_select(
                out=mdiag_f[:], in_=ones_f[:], compare_op=ALU.is_ge,
                fill=0.0, base=0, channel_multiplier=-1, pattern=[[1, 128]],
            )
            # anti-diag mask (allowed k > q): keep where p - col - 1 >= 0
            nc.gpsimd.affine_select(
                out=madiag_f[:], in_=ones_f[:], compare_op=ALU.is_ge,
                fill=0.0, base=-1, channel_multiplier=1, pattern=[[-1, 128]],
            )
            ones = constp.tile([128, 128], f16)
            mdiag = constp.tile([128, 128], f16)
            madiag = constp.tile([128, 128], f16)
            nc.vector.tensor_copy(ones[:], ones_f[:])
            nc.vector.tensor_copy(mdiag[:], mdiag_f[:])
            nc.vector.tensor_copy(madiag[:], madiag_f[:])

            def half_ap(ts, c0, c1):
                for i, (a, b) in enumerate(PIECES):
                    if a <= c0 and c1 <= b:
                        return ts[i][:, c0 - a:c1 - a]
                raise AssertionError(f"range [{c0},{c1}) crosses pieces")

            for bh in range(n_bh):
                kts, qts, vb = loaded.pop(bh)
                if bh + 1 < n_bh:
                    loaded[bh + 1] = prefetch(bh + 1)

                for g in range(NG):
                    tiles = pack_tiles(band_blocks(g))
                    pv = ps_pv.tile([128, GW], f32, tag="pv")
                    sm = ps_sum.tile([128, GW], f32, tag="sm")

                    # f32 PSUM writes max out at one bank (512 cols): split
                    # each block's pv/sm range at absolute 512 boundaries.
                    # Emission order = tile order, so per-bank first/last
                    # flags are known up front; the widest block is emitted
                    # first and fully covers every bank it touches.
                    def pieces_of(tl):
                        out = []
                        for (b, t_min, t_max), eo in tl:
                            w = (t_max - t_min + 1) * 128
                            off = (t_min - G * g) * 128
                            a = off
                            while a < off + w:
                                bnd = min(off + w, (a // 512 + 1) * 512)
                                out.append((b, a, bnd, eo + a - off))
                                a = bnd
                        return out

                    all_pieces = [pieces_of(tl) for tl in tiles]
                    flat = [p for tp in all_pieces for p in tp]
                    first_in_bank = {}
                    last_in_bank = {}
                    for i, (b, a, bnd, ea) in enumerate(flat):
                        bank = a // 512
                        first_in_bank.setdefault(bank, i)
                        last_in_bank[bank] = i
                    pidx = [0]

                    def emit_pvsm(es, tp):
                        for b, a, bnd, ea in tp:
                            i = pidx[0]
                            bank = a // 512
                            st_f = first_in_bank[bank] == i
                            sp_f = last_in_bank[bank] == i
                            nc.tensor.matmul(
                                pv[:, a:bnd], vb[:, b, :],
                                es[:, ea:ea + bnd - a],
                                start=st_f, stop=sp_f)
                            nc.tensor.matmul(
                                sm[:, a:bnd], ones[:],
                                es[:, ea:ea + bnd - a],
                                start=st_f, stop=sp_f)
                            pidx[0] += 1

                    pending = None
                    for ti, tl in enumerate(tiles):
                        stp = ps_st.tile([128, 1024], f32, tag="st")
                        es = esp.tile([128, 1024], f16, tag="es")
                        wtot = 0
                        for (b, t_min, t_max), eo in tl:
                            w = (t_max - t_min + 1) * 128
                            # split st at the stp tile's bank boundary
                            c = 0
                            while c < w:
                                cw = min(w - c, ((eo + c) // 512 + 1) * 512
                                         - (eo + c))
                                nc.tensor.matmul(
                                    stp[:, eo + c:eo + c + cw],
                                    half_ap(kts, 128 * b, 128 * b + 128),
                                    half_ap(qts, 128 * t_min + c,
                                            128 * t_min + c + cw),
                                    start=True, stop=True)
                                c += cw
                            wtot = eo + w
                        nc.scalar.activation(
                            es[:, 0:wtot], stp[:, 0:wtot], AF.Exp,
                            scale=scale)
                        for (b, t_min, t_max), eo in tl:
                            w = (t_max - t_min + 1) * 128
                            if b >= G * g:
                                nc.vector.tensor_mul(
                                    es[:, eo:eo + 128], es[:, eo:eo + 128],
                                    mdiag[:])
                            if b + WB <= G * g + G - 1:
                                nc.gpsimd.tensor_mul(
                                    es[:, eo + w - 128:eo + w],
                                    es[:, eo + w - 128:eo + w], madiag[:])
                        if pending is not None:
                            emit_pvsm(*pending)
                        pending = (es, all_pieces[ti])
                    emit_pvsm(*pending)

                    # --- normalize pre-transpose in halves (pipelines the
                    # reciprocal, multiply and store), store O^T
                    for h in range(GW // 256):
                        c = 256 * h
                        rec = outp.tile([128, 256], f32, tag=f"rec{h}")
                        nc.vector.reciprocal_approx_fast(
                            rec[:], sm[:, c:c + 256])
                        otn = outp.tile([128, 256], f16, tag=f"otn{h}")
                        nc.vector.tensor_mul(otn[:], pv[:, c:c + 256], rec[:])
                        nc.sync.dma_start(
                            out=o[bh, :, GW * g + c:GW * g + c + 256],
                            in_=otn[:])


    if not nc.is_finalized():
        nc.finalize()
    return nc


_nc = None


def _get_nc():
    global _nc
    if _nc is None:
        _nc = build_nc()
    return _nc


def make_in_maps(q, k, v):
    # [B, T, H, D] -> [B*H, T, D] fp16; q/k additionally pre-transposed
    # to [B*H, D, T] (kernel wants the [d, t] layout)
    qs = np.ascontiguousarray(
        np.asarray(q, dtype=np.float32).transpose(0, 2, 1, 3)
        .reshape(BH, T, D)).astype(np.float16)
    ks = np.ascontiguousarray(
        np.asarray(k, dtype=np.float32).transpose(0, 2, 1, 3)
        .reshape(BH, T, D)).astype(np.float16)
    vs = np.ascontiguousarray(
        np.asarray(v, dtype=np.float32).transpose(0, 2, 1, 3)
        .reshape(BH, T, D)).astype(np.float16)
    return [
        {
            "q": qs[c * BH_PER_CORE:(c + 1) * BH_PER_CORE],
            "k": ks[c * BH_PER_CORE:(c + 1) * BH_PER_CORE],
            "v": vs[c * BH_PER_CORE:(c + 1) * BH_PER_CORE],
        }
        for c in range(NCORES)
    ]


def assemble_out(results):
    # results hold O^T [n_bh, D, T] fp16 -> [BH, T, D] f32
    out = np.empty((BH, T, D), np.float32)
    for c in range(NCORES):
        ot = np.asarray(results[c]["o"], dtype=np.float32)  # [n_bh, D, T]
        out[c * BH_PER_CORE:(c + 1) * BH_PER_CORE] = ot.transpose(0, 2, 1)
    return np.ascontiguousarray(
        out.reshape(B, H, T, D).transpose(0, 2, 1, 3))


def kernel(q, k, v, window_size):
    assert int(window_size) == WINDOW
    in_maps = make_in_maps(q, k, v)
    res = run_bass_kernel_spmd(_get_nc(), in_maps, list(range(NCORES))).results
    return assemble_out(res)
